# revision 2
# baseline (speedup 1.0000x reference)
"""TRN2 Bass kernel for nn_MAD_4612794876395 (retrieval_knn).

Math: with dist = softmax_k(-||pos_d - pos_r||) and sum_k dist = 1, the
reference output collapses to
    out[b,c] = wmem@adapt_w + adapt_b + wdiff@field_b.reshape(H,C)
             + sum_h wdiff[b,h] * (date@field_w)[b, h*C+c]
where wdiff[b,h] = sum_k dist[b,k]*diff[b,k,h].  The last term is 137 GFLOP
and is computed on 8 NeuronCores, tensor-parallel over field_w's 65536
columns (64 h-values per core), as fp32r matmuls fused with a
scalar_tensor_tensor h-contraction.  The small terms are host numpy.
"""
import sys

sys.path.insert(0, "/opt/trn_rl_repo")

import numpy as np

N_DATA, F, H, C, K, B = 100000, 512, 512, 128, 8, 2048
NCORES = 8
HSH = H // NCORES          # 64 h-values per core
SH = HSH * C               # 8192 field_w cols per core
P = 128
NB = B // P                # 16 b-tiles
NS = SH // 512             # 16 n-slices of 512 cols (4 h each)

_NC = None


def _build():
    import concourse.mybir as mybir
    import concourse.tile as tile
    from concourse import bacc

    nc = bacc.Bacc(None, target_bir_lowering=False, debug=False)
    dateT = nc.dram_tensor("dateT", [F, B], mybir.dt.float32, kind="ExternalInput")
    wdiff = nc.dram_tensor("wdiff", [B, HSH], mybir.dt.float32, kind="ExternalInput")
    fw = nc.dram_tensor("fw", [F, SH], mybir.dt.float32, kind="ExternalInput")
    partial = nc.dram_tensor("partial", [B, C], mybir.dt.float32, kind="ExternalOutput")

    with tile.TileContext(nc) as tc:
        with (
            tc.tile_pool(name="const", bufs=1) as cp,
            tc.tile_pool(name="sb", bufs=1) as sb,
            tc.tile_pool(name="fwp", bufs=4) as fwp,
            tc.tile_pool(name="ps2", bufs=8, space="PSUM") as ps2,
        ):
            # dateT resident as fp32r (rounded by DVE copy)
            dr = []
            for fc in range(4):
                d_f = cp.tile([P, B], mybir.dt.float32, name=f"d_f{fc}")
                nc.sync.dma_start(d_f[:], dateT[fc * P:(fc + 1) * P, :])
                d_r = cp.tile([P, B], mybir.dt.float32r, name=f"d_r{fc}")
                nc.vector.tensor_copy(d_r[:], d_f[:])
                dr.append(d_r)
            # per-b-tile wdiff columns + accumulators
            wd, acc = [], []
            for t in range(NB):
                w_t = cp.tile([P, HSH], mybir.dt.float32, name=f"wd{t}")
                nc.sync.dma_start(w_t[:], wdiff[t * P:(t + 1) * P, :])
                wd.append(w_t)
                a_t = cp.tile([P, C], mybir.dt.float32, name=f"acc{t}")
                nc.any.memset(a_t[:], 0.0)
                acc.append(a_t)

            for n in range(NS):
                fwr = []
                for fc in range(4):
                    f_f = fwp.tile([P, 512], mybir.dt.float32, name="f_f",
                                   tag=f"f_f{fc}")
                    nc.sync.dma_start(
                        f_f[:], fw[fc * P:(fc + 1) * P, n * 512:(n + 1) * 512])
                    f_r = fwp.tile([P, 512], mybir.dt.float32r, name="f_r",
                                   tag=f"f_r{fc}")
                    nc.scalar.copy(f_r[:], f_f[:])
                    fwr.append(f_r)
                for t in range(NB):
                    g = ps2.tile([P, 512], mybir.dt.float32, name="g", tag="g")
                    for fc in range(4):
                        nc.tensor.matmul(g[:], dr[fc][:, t * P:(t + 1) * P],
                                         fwr[fc][:], start=(fc == 0), stop=(fc == 3))
                    for l in range(4):
                        hcol = 4 * n + l
                        nc.vector.scalar_tensor_tensor(
                            out=acc[t][:],
                            in0=g[:, l * C:(l + 1) * C],
                            scalar=wd[t][:, hcol:hcol + 1],
                            in1=acc[t][:],
                            op0=mybir.AluOpType.mult,
                            op1=mybir.AluOpType.add,
                        )
            for t in range(NB):
                o_t = sb.tile([P, C], mybir.dt.float32, name=f"o{t}")
                nc.vector.tensor_copy(o_t[:], acc[t][:])
                nc.sync.dma_start(partial[t * P:(t + 1) * P, :], o_t[:])
    nc.finalize()
    return nc


def kernel(idx, date, train_dates, mem, train_nns, pos_w, pos_b, field_w,
           field_b, adapt_w, adapt_b):
    global _NC
    from concourse.bass_utils import run_bass_kernel_spmd

    idx = np.asarray(idx)
    date = np.asarray(date, dtype=np.float32)
    train_dates = np.asarray(train_dates, dtype=np.float32)
    mem = np.asarray(mem, dtype=np.float32)
    train_nns = np.asarray(train_nns)
    pos_w = np.asarray(pos_w, dtype=np.float32)
    pos_b = np.asarray(pos_b, dtype=np.float32)
    field_w = np.asarray(field_w, dtype=np.float32)
    field_b = np.asarray(field_b, dtype=np.float32)
    adapt_w = np.asarray(adapt_w, dtype=np.float32)
    adapt_b = np.asarray(adapt_b, dtype=np.float32)

    # ---- host phase 1 (small): dist, wdiff, const terms ----
    refs = train_nns[idx]                                   # [B, K]
    pos_d = date @ pos_w + pos_b                            # [B, H]
    pos_r = (train_dates[refs.reshape(-1)] @ pos_w + pos_b).reshape(B, K, H)
    diff = pos_d[:, None, :] - pos_r                        # [B, K, H]
    norm = np.sqrt((diff * diff).sum(-1))                   # [B, K]
    m = norm.min(axis=1, keepdims=True)
    e = np.exp(m - norm)
    dist = e / e.sum(axis=1, keepdims=True)                 # [B, K]
    wdiff = np.einsum("bk,bkh->bh", dist, diff).astype(np.float32)
    wmem = np.einsum("bk,bkc->bc", dist, mem[refs]).astype(np.float32)
    const = wmem @ adapt_w + adapt_b + wdiff @ field_b.reshape(H, C)

    # ---- device phase 2: grad-term, TP over the 65536 dim ----
    if _NC is None:
        _NC = _build()
    dateT = np.ascontiguousarray(date.T)
    in_maps = []
    for i in range(NCORES):
        in_maps.append({
            "dateT": dateT,
            "wdiff": np.ascontiguousarray(wdiff[:, i * HSH:(i + 1) * HSH]),
            "fw": np.ascontiguousarray(field_w[:, i * SH:(i + 1) * SH]),
        })
    global _LAST_IN_MAPS
    _LAST_IN_MAPS = in_maps
    res = run_bass_kernel_spmd(_NC, in_maps, core_ids=list(range(NCORES)))
    grad_term = np.zeros((B, C), dtype=np.float32)
    for i in range(NCORES):
        grad_term += res.results[i]["partial"]
    return (const + grad_term).astype(np.float32)



# revision 9
# speedup vs baseline: 1.2234x; 1.2234x over previous
"""TRN2 Bass kernel for nn_MAD_4612794876395 (retrieval_knn).

Math: with dist = softmax_k(-||pos_d - pos_r||) and sum_k dist = 1, the
reference output collapses to
    out[b,c] = wmem@adapt_w + adapt_b + wdiff@field_b.reshape(H,C)
             + sum_h wdiff[b,h] * (date@field_w)[b, h*C+c]
where wdiff[b,h] = sum_k dist[b,k]*diff[b,k,h].  The last term is 137 GFLOP
and runs on 8 NeuronCores, tensor-parallel over the C=128 output classes
(16 c's per core, all 512 h).  field_w is re-ordered c-major on the host
so each PSUM tile [128b, 512] is ONE c over all h; the h-contraction is
then a single fused DVE tensor_tensor_reduce per tile (g * wdiff,
accumulated along the free dim into acc[:, c]).  Matmuls are bf16
(inputs pre-converted on host).  The small terms are host numpy.
"""
import sys

sys.path.insert(0, "/opt/trn_rl_repo")

import numpy as np
import ml_dtypes

N_DATA, F, H, C, K, B = 100000, 512, 512, 128, 8, 2048
NCORES = 8
CSH = C // NCORES          # 16 c-values per core
SH = CSH * H               # 8192 field_w cols per core (c-major)
P = 128
NB = B // P                # 16 b-tiles

_NC = None
_LAST_IN_MAPS = None


def _build():
    import concourse.mybir as mybir
    import concourse.tile as tile
    from concourse import bacc

    nc = bacc.Bacc(None, target_bir_lowering=False, debug=False)
    dateT = nc.dram_tensor("dateT", [F, B], mybir.dt.bfloat16, kind="ExternalInput")
    wdiff = nc.dram_tensor("wdiff", [B, H], mybir.dt.float32, kind="ExternalInput")
    fw = nc.dram_tensor("fw", [F, SH], mybir.dt.bfloat16, kind="ExternalInput")
    partial = nc.dram_tensor("partial", [B, CSH], mybir.dt.float32,
                             kind="ExternalOutput")

    with tile.TileContext(nc) as tc:
        with (
            tc.tile_pool(name="const", bufs=1) as cp,
            tc.tile_pool(name="fwp", bufs=4) as fwp,
            tc.tile_pool(name="scr", bufs=2) as scr,
            tc.tile_pool(name="ps2", bufs=8, space="PSUM") as ps2,
        ):
            # resident bf16 dateT [128, B] x 4 f-chunks
            dr = []
            for fc in range(4):
                d_t = cp.tile([P, B], mybir.dt.bfloat16, name=f"d{fc}")
                nc.sync.dma_start(d_t[:], dateT[fc * P:(fc + 1) * P, :])
                dr.append(d_t)
            # full wdiff, per b-tile [128, 512], and acc [128, CSH]
            wdt, acc = [], []
            for t in range(NB):
                w_t = cp.tile([P, H], mybir.dt.float32, name=f"wd{t}")
                nc.sync.dma_start(w_t[:], wdiff[t * P:(t + 1) * P, :])
                wdt.append(w_t)
                a_t = cp.tile([P, CSH], mybir.dt.float32, name=f"acc{t}")
                acc.append(a_t)

            for c in range(CSH):
                fwr = []
                for fc in range(4):
                    f_t = fwp.tile([P, H], mybir.dt.bfloat16, name="f",
                                   tag=f"f{fc}")
                    nc.sync.dma_start(
                        f_t[:], fw[fc * P:(fc + 1) * P, c * H:(c + 1) * H])
                    fwr.append(f_t)
                for t in range(NB):
                    g = ps2.tile([P, H], mybir.dt.float32, name="g", tag="g")
                    for fc in range(4):
                        nc.tensor.matmul(g[:], dr[fc][:, t * P:(t + 1) * P],
                                         fwr[fc][:], start=(fc == 0), stop=(fc == 3))
                    # acc[t][:, c] = sum_h g[b, h] * wdiff[b, h]
                    waste = scr.tile([P, 1], mybir.dt.float32, name="w", tag="w")
                    nc.vector.scalar_tensor_tensor(
                        out=waste[:].broadcast_to((P, H)),
                        in0=g[:],
                        scalar=1.0,
                        in1=wdt[t][:],
                        op0=mybir.AluOpType.mult,
                        op1=mybir.AluOpType.mult,
                        accum_out=acc[t][:, c:c + 1],
                    )
            for t in range(NB):
                nc.sync.dma_start(partial[t * P:(t + 1) * P, :], acc[t][:])
    nc.finalize()
    return nc


def kernel(idx, date, train_dates, mem, train_nns, pos_w, pos_b, field_w,
           field_b, adapt_w, adapt_b):
    global _NC, _LAST_IN_MAPS
    from concourse.bass_utils import run_bass_kernel_spmd

    idx = np.asarray(idx)
    date = np.asarray(date, dtype=np.float32)
    train_dates = np.asarray(train_dates, dtype=np.float32)
    mem = np.asarray(mem, dtype=np.float32)
    train_nns = np.asarray(train_nns)
    pos_w = np.asarray(pos_w, dtype=np.float32)
    pos_b = np.asarray(pos_b, dtype=np.float32)
    field_w = np.asarray(field_w, dtype=np.float32)
    field_b = np.asarray(field_b, dtype=np.float32)
    adapt_w = np.asarray(adapt_w, dtype=np.float32)
    adapt_b = np.asarray(adapt_b, dtype=np.float32)

    # ---- host phase 1 (small): dist, wdiff, const terms ----
    refs = train_nns[idx]                                   # [B, K]
    pos_d = date @ pos_w + pos_b                            # [B, H]
    pos_r = (train_dates[refs.reshape(-1)] @ pos_w + pos_b).reshape(B, K, H)
    diff = pos_d[:, None, :] - pos_r                        # [B, K, H]
    norm = np.sqrt((diff * diff).sum(-1))                   # [B, K]
    m = norm.min(axis=1, keepdims=True)
    e = np.exp(m - norm)
    dist = e / e.sum(axis=1, keepdims=True)                 # [B, K]
    wdiff = np.einsum("bk,bkh->bh", dist, diff).astype(np.float32)
    wmem = np.einsum("bk,bkc->bc", dist, mem[refs]).astype(np.float32)
    const = wmem @ adapt_w + adapt_b + wdiff @ field_b.reshape(H, C)

    # ---- device phase 2: grad-term, TP over the C=128 classes ----
    if _NC is None:
        _NC = _build()
    dateT_bf = np.ascontiguousarray(date.T).astype(ml_dtypes.bfloat16)
    # c-major column gather: core i gets cols [c*1 for c in its 16 c's] x h
    fw3 = field_w.reshape(F, H, C)
    in_maps = []
    for i in range(NCORES):
        cols = np.arange(i * CSH, (i + 1) * CSH)
        # shard[f, c_local*H + h] = field_w[f, h*C + c]
        shard = np.ascontiguousarray(
            fw3[:, :, cols].transpose(0, 2, 1).reshape(F, SH)
        ).astype(ml_dtypes.bfloat16)
        in_maps.append({
            "dateT": dateT_bf,
            "wdiff": wdiff,
            "fw": shard,
        })
    _LAST_IN_MAPS = in_maps
    res = run_bass_kernel_spmd(_NC, in_maps, core_ids=list(range(NCORES)))
    grad_term = np.concatenate(
        [res.results[i]["partial"] for i in range(NCORES)], axis=1)
    return (const + grad_term).astype(np.float32)


# revision 11
# speedup vs baseline: 1.2503x; 1.0219x over previous
"""TRN2 Bass kernel for nn_MAD_4612794876395 (retrieval_knn).

Math: with dist = softmax_k(-||pos_d - pos_r||) and sum_k dist = 1, the
reference output collapses to
    out[b,c] = wmem@adapt_w + adapt_b + wdiff@field_b.reshape(H,C)
             + sum_h wdiff[b,h] * (date@field_w)[b, h*C+c]
where wdiff[b,h] = sum_k dist[b,k]*diff[b,k,h].  The last term is 137 GFLOP
and runs on 8 NeuronCores, tensor-parallel over the C=128 output classes
(16 c's per core, all 512 h).  field_w is re-ordered c-major on the host
so each PSUM tile [128b, 512] is ONE c over all h; the h-contraction is
then a single fused DVE tensor_tensor_reduce per tile (g * wdiff,
accumulated along the free dim into acc[:, c]).  Matmuls are bf16
(inputs pre-converted on host).  The small terms are host numpy.
"""
import sys

sys.path.insert(0, "/opt/trn_rl_repo")

import numpy as np
import ml_dtypes

N_DATA, F, H, C, K, B = 100000, 512, 512, 128, 8, 2048
NCORES = 8
CSH = C // NCORES          # 16 c-values per core
SH = CSH * H               # 8192 field_w cols per core (c-major)
P = 128
NB = B // P                # 16 b-tiles

_NC = None
_LAST_IN_MAPS = None


def _build():
    import concourse.mybir as mybir
    import concourse.tile as tile
    from concourse import bacc

    nc = bacc.Bacc(None, target_bir_lowering=False, debug=False)
    dateT = nc.dram_tensor("dateT", [F, B], mybir.dt.bfloat16, kind="ExternalInput")
    wdiff = nc.dram_tensor("wdiff", [B, H], mybir.dt.bfloat16, kind="ExternalInput")
    fw = nc.dram_tensor("fw", [F, SH], mybir.dt.bfloat16, kind="ExternalInput")
    partial = nc.dram_tensor("partial", [B, CSH], mybir.dt.float32,
                             kind="ExternalOutput")

    with tile.TileContext(nc) as tc:
        with (
            tc.tile_pool(name="const", bufs=1) as cp,
            tc.tile_pool(name="fwp", bufs=6) as fwp,
            tc.tile_pool(name="scr", bufs=2) as scr,
            tc.tile_pool(name="ps2", bufs=8, space="PSUM") as ps2,
        ):
            # dateT fc0 + first fw slice first so the PE starts ASAP;
            # wdiff rides the ACT hw-dge ring in parallel.
            dr = [cp.tile([P, B], mybir.dt.bfloat16, name=f"d{fc}")
                  for fc in range(4)]
            nc.sync.dma_start(dr[0][:], dateT[0:P, :])
            f0 = []
            for fc in range(4):
                f_t = fwp.tile([P, H], mybir.dt.bfloat16, name="f", tag=f"f{fc}")
                nc.sync.dma_start(f_t[:], fw[fc * P:(fc + 1) * P, 0:H])
                f0.append(f_t)
            for fc in range(1, 4):
                nc.sync.dma_start(dr[fc][:], dateT[fc * P:(fc + 1) * P, :])
            wdt, acc = [], []
            for t in range(NB):
                w_t = cp.tile([P, H], mybir.dt.bfloat16, name=f"wd{t}")
                nc.scalar.dma_start(w_t[:], wdiff[t * P:(t + 1) * P, :])
                wdt.append(w_t)
                a_t = cp.tile([P, CSH], mybir.dt.float32, name=f"acc{t}")
                acc.append(a_t)

            for c in range(CSH):
                if c == 0:
                    fwr = f0
                else:
                    fwr = []
                    for fc in range(4):
                        f_t = fwp.tile([P, H], mybir.dt.bfloat16, name="f",
                                       tag=f"f{fc}")
                        nc.sync.dma_start(
                            f_t[:], fw[fc * P:(fc + 1) * P, c * H:(c + 1) * H])
                        fwr.append(f_t)
                for t in range(NB):
                    g = ps2.tile([P, H], mybir.dt.float32, name="g", tag="g")
                    for fc in range(4):
                        nc.tensor.matmul(g[:], dr[fc][:, t * P:(t + 1) * P],
                                         fwr[fc][:], start=(fc == 0), stop=(fc == 3))
                    # acc[t][:, c] = sum_h g[b, h] * wdiff[b, h]
                    waste = scr.tile([P, 1], mybir.dt.float32, name="w", tag="w")
                    nc.vector.scalar_tensor_tensor(
                        out=waste[:].broadcast_to((P, H)),
                        in0=g[:],
                        scalar=1.0,
                        in1=wdt[t][:],
                        op0=mybir.AluOpType.mult,
                        op1=mybir.AluOpType.mult,
                        accum_out=acc[t][:, c:c + 1],
                    )
            for t in range(NB):
                nc.sync.dma_start(partial[t * P:(t + 1) * P, :], acc[t][:])
    nc.finalize()
    return nc


def kernel(idx, date, train_dates, mem, train_nns, pos_w, pos_b, field_w,
           field_b, adapt_w, adapt_b):
    global _NC, _LAST_IN_MAPS
    from concourse.bass_utils import run_bass_kernel_spmd

    idx = np.asarray(idx)
    date = np.asarray(date, dtype=np.float32)
    train_dates = np.asarray(train_dates, dtype=np.float32)
    mem = np.asarray(mem, dtype=np.float32)
    train_nns = np.asarray(train_nns)
    pos_w = np.asarray(pos_w, dtype=np.float32)
    pos_b = np.asarray(pos_b, dtype=np.float32)
    field_w = np.asarray(field_w, dtype=np.float32)
    field_b = np.asarray(field_b, dtype=np.float32)
    adapt_w = np.asarray(adapt_w, dtype=np.float32)
    adapt_b = np.asarray(adapt_b, dtype=np.float32)

    # ---- host phase 1 (small): dist, wdiff, const terms ----
    refs = train_nns[idx]                                   # [B, K]
    pos_d = date @ pos_w + pos_b                            # [B, H]
    pos_r = (train_dates[refs.reshape(-1)] @ pos_w + pos_b).reshape(B, K, H)
    diff = pos_d[:, None, :] - pos_r                        # [B, K, H]
    norm = np.sqrt((diff * diff).sum(-1))                   # [B, K]
    m = norm.min(axis=1, keepdims=True)
    e = np.exp(m - norm)
    dist = e / e.sum(axis=1, keepdims=True)                 # [B, K]
    wdiff = np.einsum("bk,bkh->bh", dist, diff).astype(np.float32)
    wmem = np.einsum("bk,bkc->bc", dist, mem[refs]).astype(np.float32)
    const = wmem @ adapt_w + adapt_b + wdiff @ field_b.reshape(H, C)

    # ---- device phase 2: grad-term, TP over the C=128 classes ----
    if _NC is None:
        _NC = _build()
    dateT_bf = np.ascontiguousarray(date.T).astype(ml_dtypes.bfloat16)
    # c-major column gather: core i gets cols [c*1 for c in its 16 c's] x h
    fw3 = field_w.reshape(F, H, C)
    in_maps = []
    for i in range(NCORES):
        cols = np.arange(i * CSH, (i + 1) * CSH)
        # shard[f, c_local*H + h] = field_w[f, h*C + c]
        shard = np.ascontiguousarray(
            fw3[:, :, cols].transpose(0, 2, 1).reshape(F, SH)
        ).astype(ml_dtypes.bfloat16)
        in_maps.append({
            "dateT": dateT_bf,
            "wdiff": wdiff.astype(ml_dtypes.bfloat16),
            "fw": shard,
        })
    _LAST_IN_MAPS = in_maps
    res = run_bass_kernel_spmd(_NC, in_maps, core_ids=list(range(NCORES)))
    grad_term = np.concatenate(
        [res.results[i]["partial"] for i in range(NCORES)], axis=1)
    return (const + grad_term).astype(np.float32)


# revision 13
# speedup vs baseline: 1.2813x; 1.0249x over previous
"""TRN2 Bass kernel for nn_MAD_4612794876395 (retrieval_knn).

Math: with dist = softmax_k(-||pos_d - pos_r||) and sum_k dist = 1, the
reference output collapses to
    out[b,c] = wmem@adapt_w + adapt_b + wdiff@field_b.reshape(H,C)
             + sum_h wdiff[b,h] * (date@field_w)[b, h*C+c]
where wdiff[b,h] = sum_k dist[b,k]*diff[b,k,h].  The last term is 137 GFLOP
and runs on 8 NeuronCores, tensor-parallel over the C=128 output classes
(16 c's per core, all 512 h).  field_w is re-ordered c-major on the host
so each PSUM tile [128b, 512] is ONE c over all h; the h-contraction is
then a single fused DVE tensor_tensor_reduce per tile (g * wdiff,
accumulated along the free dim into acc[:, c]).  Matmuls are bf16
(inputs pre-converted on host).  The small terms are host numpy.
"""
import sys

sys.path.insert(0, "/opt/trn_rl_repo")

import numpy as np
import ml_dtypes

N_DATA, F, H, C, K, B = 100000, 512, 512, 128, 8, 2048
NCORES = 8
CSH = C // NCORES          # 16 c-values per core
SH = CSH * H               # 8192 field_w cols per core (c-major)
P = 128
NB = B // P                # 16 b-tiles

_NC = None
_LAST_IN_MAPS = None


def _build():
    import concourse.mybir as mybir
    import concourse.tile as tile
    from concourse import bacc

    nc = bacc.Bacc(None, target_bir_lowering=False, debug=False)
    dateT = nc.dram_tensor("dateT", [F, B], mybir.dt.bfloat16, kind="ExternalInput")
    wdiff = nc.dram_tensor("wdiff", [B, H], mybir.dt.bfloat16, kind="ExternalInput")
    fw = nc.dram_tensor("fw", [F, SH], mybir.dt.bfloat16, kind="ExternalInput")
    partial = nc.dram_tensor("partial", [B, CSH], mybir.dt.float32,
                             kind="ExternalOutput")

    with tile.TileContext(nc) as tc:
        with (
            tc.tile_pool(name="const", bufs=1) as cp,
            tc.tile_pool(name="fwp", bufs=6) as fwp,
            tc.tile_pool(name="scr", bufs=2) as scr,
            tc.tile_pool(name="ps2", bufs=8, space="PSUM") as ps2,
        ):
            # dateT fc0 + first fw slice first so the PE starts ASAP;
            # wdiff rides the ACT hw-dge ring in parallel.
            dr = [cp.tile([P, B], mybir.dt.bfloat16, name=f"d{fc}")
                  for fc in range(4)]
            f0 = []
            for fc in range(4):
                f_t = fwp.tile([P, H], mybir.dt.bfloat16, name="f", tag=f"f{fc}")
                nc.sync.dma_start(f_t[:], fw[fc * P:(fc + 1) * P, 0:H])
                f0.append(f_t)
            nc.sync.dma_start(dr[0][:], dateT[0:P, :])
            for fc in range(1, 4):
                nc.scalar.dma_start(dr[fc][:], dateT[fc * P:(fc + 1) * P, :])
            wdt, acc = [], []
            for t in range(NB):
                w_t = cp.tile([P, H], mybir.dt.bfloat16, name=f"wd{t}")
                nc.scalar.dma_start(w_t[:], wdiff[t * P:(t + 1) * P, :])
                wdt.append(w_t)
                a_t = cp.tile([P, CSH], mybir.dt.float32, name=f"acc{t}")
                acc.append(a_t)

            for c in range(CSH):
                if c == 0:
                    fwr = f0
                else:
                    fwr = []
                    for fc in range(4):
                        f_t = fwp.tile([P, H], mybir.dt.bfloat16, name="f",
                                       tag=f"f{fc}")
                        nc.sync.dma_start(
                            f_t[:], fw[fc * P:(fc + 1) * P, c * H:(c + 1) * H])
                        fwr.append(f_t)
                for t in range(NB):
                    g = ps2.tile([P, H], mybir.dt.float32, name="g", tag="g")
                    for fc in range(4):
                        nc.tensor.matmul(g[:], dr[fc][:, t * P:(t + 1) * P],
                                         fwr[fc][:], start=(fc == 0), stop=(fc == 3))
                    # acc[t][:, c] = sum_h g[b, h] * wdiff[b, h]
                    waste = scr.tile([P, 1], mybir.dt.float32, name="w", tag="w")
                    nc.vector.scalar_tensor_tensor(
                        out=waste[:].broadcast_to((P, H)),
                        in0=g[:],
                        scalar=1.0,
                        in1=wdt[t][:],
                        op0=mybir.AluOpType.mult,
                        op1=mybir.AluOpType.mult,
                        accum_out=acc[t][:, c:c + 1],
                    )
            for t in range(NB):
                eng = nc.sync if t % 2 == 0 else nc.scalar
                eng.dma_start(partial[t * P:(t + 1) * P, :], acc[t][:])
    nc.finalize()
    return nc


def kernel(idx, date, train_dates, mem, train_nns, pos_w, pos_b, field_w,
           field_b, adapt_w, adapt_b):
    global _NC, _LAST_IN_MAPS
    from concourse.bass_utils import run_bass_kernel_spmd

    idx = np.asarray(idx)
    date = np.asarray(date, dtype=np.float32)
    train_dates = np.asarray(train_dates, dtype=np.float32)
    mem = np.asarray(mem, dtype=np.float32)
    train_nns = np.asarray(train_nns)
    pos_w = np.asarray(pos_w, dtype=np.float32)
    pos_b = np.asarray(pos_b, dtype=np.float32)
    field_w = np.asarray(field_w, dtype=np.float32)
    field_b = np.asarray(field_b, dtype=np.float32)
    adapt_w = np.asarray(adapt_w, dtype=np.float32)
    adapt_b = np.asarray(adapt_b, dtype=np.float32)

    # ---- host phase 1 (small): dist, wdiff, const terms ----
    refs = train_nns[idx]                                   # [B, K]
    pos_d = date @ pos_w + pos_b                            # [B, H]
    pos_r = (train_dates[refs.reshape(-1)] @ pos_w + pos_b).reshape(B, K, H)
    diff = pos_d[:, None, :] - pos_r                        # [B, K, H]
    norm = np.sqrt((diff * diff).sum(-1))                   # [B, K]
    m = norm.min(axis=1, keepdims=True)
    e = np.exp(m - norm)
    dist = e / e.sum(axis=1, keepdims=True)                 # [B, K]
    wdiff = np.einsum("bk,bkh->bh", dist, diff).astype(np.float32)
    wmem = np.einsum("bk,bkc->bc", dist, mem[refs]).astype(np.float32)
    const = wmem @ adapt_w + adapt_b + wdiff @ field_b.reshape(H, C)

    # ---- device phase 2: grad-term, TP over the C=128 classes ----
    if _NC is None:
        _NC = _build()
    dateT_bf = np.ascontiguousarray(date.T).astype(ml_dtypes.bfloat16)
    # c-major column gather: core i gets cols [c*1 for c in its 16 c's] x h
    fw3 = field_w.reshape(F, H, C)
    in_maps = []
    for i in range(NCORES):
        cols = np.arange(i * CSH, (i + 1) * CSH)
        # shard[f, c_local*H + h] = field_w[f, h*C + c]
        shard = np.ascontiguousarray(
            fw3[:, :, cols].transpose(0, 2, 1).reshape(F, SH)
        ).astype(ml_dtypes.bfloat16)
        in_maps.append({
            "dateT": dateT_bf,
            "wdiff": wdiff.astype(ml_dtypes.bfloat16),
            "fw": shard,
        })
    _LAST_IN_MAPS = in_maps
    res = run_bass_kernel_spmd(_NC, in_maps, core_ids=list(range(NCORES)))
    grad_term = np.concatenate(
        [res.results[i]["partial"] for i in range(NCORES)], axis=1)
    return (const + grad_term).astype(np.float32)
